# revision 16
# baseline (speedup 1.0000x reference)
"""Trainium2 Bass kernel for nn_MHA_9603546874182.

Causal MHA: qkv proj + rope(32) + causal attention + out proj.
B=4, T=1024, C=2048, H=32, hd=64.

Sharding: 8-way tensor parallel over heads (4 heads / core).
Each core computes qkv for its 4 heads (column-parallel), rope,
causal attention, and a row-parallel partial of the output
projection (bf16 partials). Host sums the 8 partials (+ bias; the
v-bias is folded into the host-side output bias since
sum_s p[s] (v[s]+bv) = (sum p v) + bv after softmax).

Device pipeline (per core), software-pipelined across batches
(822us baseline -> 403us):
  tick (b, m8):
    phase1(b, m8):  v MMs (bf16) -> Vp;  q/k MMs -> +bias (DVE, fp32) ->
                    rope (DVE, bf16) -> PE-transpose pairs -> QT/KT (bf16)
    phase2:         one head-pair unit (2 heads on SBUF partition halves;
                    their K=64 score MMs run concurrently in different PE
                    row groups via auto tile_position). Scores^T skip the
                    fully-masked column block of diagonal-crossing s-tiles;
                    exp on ACT over live columns only; causal masking = one
                    triangular bf16 multiply on the 128-wide diagonal band
                    of the probabilities (DVE); AV+rowsum via [V|1] lhsT;
                    reciprocal_approx_fast (base-0 SBUF bounce: the custom
                    DVE op mis-addresses base_partition!=0 inputs) +
                    gpsimd partition_broadcast (sole gpsimd op kind --
                    mixing op kinds there costs a ~5.6us LOAD_LIB swap) +
                    normalize -> ctxT (bf16). qb0 units run inside their
                    own batch's phase1 ticks (they only need token tiles
                    0-3); qb1 units + all phase3 run in the next batch's
                    first 4 ticks, keeping the end-of-kernel drain short.
    phase3(bb, mm): out partial = ctxT.T @ w2 (bf16), psum -> DVE/ACT
                    copy (alternating) -> DRAM bf16
"""

import numpy as np

B, T, C, H = 4, 1024, 2048, 32
HD = C // H          # 64
NCORES = 8
HPC = H // NCORES    # 4 heads per core
SC = HPC * HD        # 256 shard channels
NTOK = B * T         # 4096
KT16 = C // 128      # 16 k tiles
MT = NTOK // 128     # 32 token tiles
MPB = T // 128       # 8 token tiles per batch
ROT = 32
NEG = -1.0e9

_CACHE = {}


def _build_nc():
    import concourse.bass as bass
    import concourse.mybir as mybir
    import concourse.tile as tile
    from concourse import bacc

    f32 = mybir.dt.float32
    f32r = mybir.dt.float32r
    bf16 = mybir.dt.bfloat16

    nc = bacc.Bacc("TRN2")

    xt_d = nc.dram_tensor("xt", [128, KT16, MT, 128], bf16, kind="ExternalInput")
    wq_d = nc.dram_tensor("wq", [128, KT16, 3 * SC], bf16, kind="ExternalInput")
    br_d = nc.dram_tensor("br", [128, 2 * SC], f32, kind="ExternalInput")
    c1_d = nc.dram_tensor("c1", [128, MPB, SC], bf16, kind="ExternalInput")
    c2_d = nc.dram_tensor("c2", [128, MPB, SC], bf16, kind="ExternalInput")
    tri_d = nc.dram_tensor("tri", [128, 128], bf16, kind="ExternalInput")
    id_d = nc.dram_tensor("id", [128, 128], bf16, kind="ExternalInput")
    w2_d = nc.dram_tensor("w2", [128, 2, C], bf16, kind="ExternalInput")
    out_d = nc.dram_tensor("out", [MT, 128, C], bf16, kind="ExternalOutput")

    with tile.TileContext(nc) as tc:
        with (
            tc.tile_pool(name="const", bufs=1) as const,
            tc.tile_pool(name="bigp", bufs=2) as bigp,
            tc.tile_pool(name="xp", bufs=4) as xp,
            tc.tile_pool(name="qkvp", bufs=4) as qkvp,
            tc.tile_pool(name="rtp", bufs=3) as rtp,
            tc.tile_pool(name="ptp", bufs=6) as ptp,
            tc.tile_pool(name="otp", bufs=3) as otp,
            tc.tile_pool(name="rsp", bufs=3) as rsp,
            tc.tile_pool(name="crp", bufs=3) as crp,
            tc.tile_pool(name="pqk", bufs=2, space="PSUM") as pqk,
            tc.tile_pool(name="pvb", bufs=1, space="PSUM") as pvb,
            tc.tile_pool(name="stp", bufs=3, space="PSUM") as stpp,
            tc.tile_pool(name="pcp", bufs=2, space="PSUM") as pcp,
        ):
            wq = const.tile([128, KT16, 3 * SC], bf16)
            nc.sync.dma_start(wq[:], wq_d[:])
            w2 = const.tile([128, 2, C], bf16)
            nc.sync.dma_start(w2[:], w2_d[:])
            br = const.tile([128, 2 * SC], f32)
            nc.sync.dma_start(br[:], br_d[:])
            c1 = const.tile([128, MPB, SC], bf16)
            nc.sync.dma_start(c1[:], c1_d[:])
            c2 = const.tile([128, MPB, SC], bf16)
            nc.sync.dma_start(c2[:], c2_d[:])
            tri = const.tile([128, 128], bf16)
            nc.sync.dma_start(tri[:], tri_d[:])
            ident = const.tile([128, 128], bf16)
            nc.sync.dma_start(ident[:], id_d[:])

            # Per-batch double-buffered state
            QTs, KTs, Vps, CTs = {}, {}, {}, {}

            def get_batch_tiles(b):
                if b not in QTs:
                    QTs[b] = bigp.tile([128, 2, T], bf16, tag="qt",
                                       name=f"qt{b}")
                    KTs[b] = bigp.tile([128, 2, T], bf16, tag="kt",
                                       name=f"kt{b}")
                    Vps[b] = bigp.tile([128, MPB, HPC, HD + 1], bf16,
                                       tag="vp", name=f"vp{b}")
                    CTs[b] = bigp.tile([128, 2, T], bf16, tag="ct",
                                       name=f"ct{b}")
                    nc.vector.memset(Vps[b][:, :, :, HD:HD + 1], 1.0)
                return QTs[b], KTs[b], Vps[b], CTs[b]

            def phase1(b, m8):
                QT, KTt, Vp, _ = get_batch_tiles(b)
                m = b * MPB + m8
                xt = xp.tile([128, KT16, 128], bf16)
                nc.sync.dma_start(xt[:], xt_d[:, :, m, :])
                # v matmuls (no bias: folded into host output bias)
                psB = pvb.tile([128, SC], f32, tag="vtr", name="psB")
                for k in range(KT16):
                    nc.tensor.matmul(
                        psB[:], xt[:, k, :], wq[:, k, 2 * SC:3 * SC],
                        start=(k == 0), stop=(k == KT16 - 1))
                nc.vector.tensor_copy(Vp[:, m8, :, 0:HD], psB[:].rearrange(
                    "p (h d) -> p h d", h=HPC))
                # q/k matmuls
                psA = pqk.tile([128, 512], f32)
                for k in range(KT16):
                    nc.tensor.matmul(
                        psA[:], xt[:, k, :], wq[:, k, 0:512],
                        start=(k == 0), stop=(k == KT16 - 1))
                qkv = qkvp.tile([128, 512], bf16)
                nc.vector.tensor_add(qkv[:], psA[:], br[:])
                # rope on DVE (bf16 2x mode; gpsimd is reserved for
                # partition_broadcast — mixing op kinds there causes a
                # ~5.6us LOAD_LIB stall per switch)
                c1v = c1[:, m8, :].rearrange("p (h d) -> p h d", h=HPC)
                c2v = c2[:, m8, :].rearrange("p (h d) -> p h d", h=HPC)
                for base in (0, 256):
                    sec = qkv[:, base:base + 256].rearrange(
                        "p (h d) -> p h d", h=HPC)
                    rt = rtp.tile([128, 256], bf16)
                    rtv = rt.rearrange("p (h d) -> p h d", h=HPC)
                    nc.vector.tensor_mul(
                        rtv[:, :, 0:16], sec[:, :, 16:32], c2v[:, :, 0:16])
                    nc.vector.tensor_mul(
                        rtv[:, :, 16:32], sec[:, :, 0:16], c2v[:, :, 16:32])
                    nc.vector.tensor_mul(sec[:], sec[:], c1v)
                    nc.vector.tensor_add(
                        sec[:, :, 0:ROT], sec[:, :, 0:ROT], rtv[:, :, 0:ROT])
                # transpose q/k -> QT/KT (bf16), two 128-chunks per psum
                # tile so one cast moves both
                for base, dst in ((0, QT), (256, KTt)):
                    tp = pvb.tile([128, 2, 128], bf16, tag="vtr", name="tp")
                    for ci in range(2):
                        nc.tensor.transpose(
                            tp[:, ci, :],
                            qkv[:, base + ci * 128: base + (ci + 1) * 128],
                            ident)
                    nc.vector.tensor_copy(
                        dst[:, 0:2, m8 * 128:(m8 + 1) * 128], tp[:])

            def phase2(bb, u):
                QT, KTt, Vp, ctxT = get_batch_tiles(bb)
                qb, j = u // 2, u % 2
                hA, hB = 2 * j, 2 * j + 1
                nst = 4 * (qb + 1)
                qsl = slice(qb * 512, (qb + 1) * 512)
                pctA = pcp.tile([HD + 1, 512], f32, tag="pc")
                pctB = pcp.tile([HD + 1, 512], f32, tag="pc")
                for st in range(nst):
                    r = st - 4 * qb
                    # columns [0, 128r) of this s-tile are fully in the
                    # future for every s row — skip them entirely; only the
                    # diagonal 128-band needs the triangular keep-mask
                    lo = 128 * r if r > 0 else 0
                    stA = stpp.tile([128, 512], f32, tag="st")
                    stB = stpp.tile([128, 512], f32, tag="st")
                    ssl = slice(st * 128, (st + 1) * 128)
                    qls = slice(qb * 512 + lo, (qb + 1) * 512)
                    nc.tensor.matmul(
                        stA[:, lo:512], KTt[0:64, j, ssl], QT[0:64, j, qls],
                        start=True, stop=True)
                    nc.tensor.matmul(
                        stB[:, lo:512], KTt[64:128, j, ssl], QT[64:128, j, qls],
                        start=True, stop=True)
                    ptA = ptp.tile([128, 512], bf16, tag="pt")
                    ptB = ptp.tile([128, 512], bf16, tag="pt")
                    nc.scalar.activation(
                        ptA[:, lo:512], stA[:, lo:512],
                        mybir.ActivationFunctionType.Exp)
                    nc.scalar.activation(
                        ptB[:, lo:512], stB[:, lo:512],
                        mybir.ActivationFunctionType.Exp)
                    if r >= 0:
                        nc.vector.tensor_mul(
                            ptA[:, lo:lo + 128], ptA[:, lo:lo + 128], tri[:])
                        nc.vector.tensor_mul(
                            ptB[:, lo:lo + 128], ptB[:, lo:lo + 128], tri[:])
                    nc.tensor.matmul(
                        pctA[:, lo:512], Vp[:, st, hA, :], ptA[:, lo:512],
                        start=(st == 0), stop=(st == nst - 1),
                        skip_group_check=True)
                    nc.tensor.matmul(
                        pctB[:, lo:512], Vp[:, st, hB, :], ptB[:, lo:512],
                        start=(st == 0), stop=(st == nst - 1),
                        skip_group_check=True)
                for pct, p0 in ((pctA, 0), (pctB, 64)):
                    # reciprocal_approx_fast mis-addresses base_partition!=0
                    # inputs — bounce the rowsum row to a base-0 SBUF tile.
                    # Copy ctx rows out too so the psum bank frees ~2us
                    # earlier (phase3's po tiles wait on these banks).
                    rs0 = rsp.tile([1, 512], f32, tag="rs0")
                    nc.vector.tensor_copy(rs0[:], pct[HD:HD + 1, :])
                    cr = crp.tile([HD, 512], f32, tag="cr")
                    nc.vector.tensor_copy(cr[:], pct[0:HD, :])
                    rs = rsp.tile([1, 512], f32, tag="rs")
                    nc.vector.reciprocal_approx_fast(rs[:], rs0[:])
                    rsb = rsp.tile([HD, 512], f32, tag="rsb")
                    nc.gpsimd.partition_broadcast(rsb[:], rs[:])
                    nc.vector.tensor_mul(
                        ctxT[p0:p0 + 64, j, qsl], cr[:], rsb[:])

            def phase3(bb, mm):
                _, _, _, ctxT = get_batch_tiles(bb)
                m = bb * MPB + mm
                msl = slice(mm * 128, (mm + 1) * 128)
                ot = otp.tile([128, C], bf16)
                for n in range(4):
                    po = pcp.tile([128, 512], f32, tag="pc")
                    for jj in range(2):
                        nc.tensor.matmul(
                            po[:], ctxT[:, jj, msl],
                            w2[:, jj, n * 512:(n + 1) * 512],
                            start=(jj == 0), stop=(jj == 1))
                    if n % 2 == 0:
                        nc.vector.tensor_copy(ot[:, n * 512:(n + 1) * 512], po[:])
                    else:
                        nc.scalar.copy(ot[:, n * 512:(n + 1) * 512], po[:])
                nc.sync.dma_start(out_d[m, :, :], ot[:])

            # qb0 units only need the first 4 token tiles of their own
            # batch -> run them inside batch b's phase1 ticks; qb1 units and
            # all phase3 chunks run in the first 4 ticks of the next batch,
            # shrinking the end-of-kernel drain to 4 ticks.
            for tick in range(B * MPB + 4):
                b, m8 = divmod(tick, MPB)
                if b < B:
                    phase1(b, m8)
                    if m8 == 5:
                        phase2(b, 0)
                    elif m8 == 6:
                        phase2(b, 1)
                if 1 <= b <= B and m8 in (0, 1):
                    phase2(b - 1, 2 + m8)
                if 1 <= b <= B and m8 < 4:
                    phase3(b - 1, 2 * m8)
                    phase3(b - 1, 2 * m8 + 1)

    nc.finalize()
    return nc


def _host_prep(x, rope, Wqkv_w, Wqkv_b, out_w):
    """Build per-core input maps (bf16 partition-first layouts)."""
    import ml_dtypes
    bf16 = ml_dtypes.bfloat16

    xf = np.ascontiguousarray(x.reshape(NTOK, C)).astype(np.float32)
    # xt[p, k, m, t] = x[m*128+t, k*128+p]
    xt = np.ascontiguousarray(
        xf.reshape(MT, 128, KT16, 128).transpose(3, 2, 0, 1)).astype(bf16)

    # rope tables (position within a batch: t = 0..1023)
    cos = rope[:, :, 0].astype(np.float32)   # [T, 16]
    sin = rope[:, :, 1].astype(np.float32)
    C1h = np.ones((T, HD), np.float32)
    C1h[:, 0:16] = cos
    C1h[:, 16:32] = cos
    C2h = np.zeros((T, HD), np.float32)
    C2h[:, 0:16] = -sin
    C2h[:, 16:32] = sin
    C1 = np.tile(C1h, (1, HPC))              # [T, 256]
    C2 = np.tile(C2h, (1, HPC))
    # c1[p, q, j] = C1[q*128+p, j]
    c1 = np.ascontiguousarray(
        C1.reshape(MPB, 128, SC).transpose(1, 0, 2)).astype(bf16)
    c2 = np.ascontiguousarray(
        C2.reshape(MPB, 128, SC).transpose(1, 0, 2)).astype(bf16)

    # triangular keep-mask for the diagonal 128-band: tri[p, y] = (y >= p)
    yy = np.arange(128)[None, :]
    pp = np.arange(128)[:, None]
    tri = (yy >= pp).astype(bf16)
    idm = np.eye(128, dtype=np.float32).astype(bf16)

    scale = np.float32(1.0 / np.sqrt(HD))
    in_maps = []
    for g in range(NCORES):
        hs = g * SC
        Wq = Wqkv_w[hs:hs + SC, :].astype(np.float32) * scale
        Wk = Wqkv_w[C + hs:C + hs + SC, :].astype(np.float32)
        Wv = Wqkv_w[2 * C + hs:2 * C + hs + SC, :].astype(np.float32)
        Wsh = np.concatenate([Wq, Wk, Wv], axis=0)          # [768, 2048]
        # wq[p, k, j] = Wsh[j, k*128+p]
        wqa = np.ascontiguousarray(
            Wsh.T.reshape(KT16, 128, 3 * SC).transpose(1, 0, 2)).astype(bf16)
        bq = Wqkv_b[hs:hs + SC].astype(np.float32) * scale
        bk = Wqkv_b[C + hs:C + hs + SC].astype(np.float32)
        bsh = np.concatenate([bq, bk])
        bra = np.ascontiguousarray(
            np.broadcast_to(bsh, (128, 2 * SC))).astype(np.float32)
        # w2[p, j, o] = out_w[o, g*256 + j*128 + p]
        w2a = np.ascontiguousarray(
            out_w[:, hs:hs + SC].astype(np.float32).T.reshape(
                2, 128, C).transpose(1, 0, 2)).astype(bf16)
        in_maps.append({
            "xt": xt, "wq": wqa, "br": bra, "c1": c1, "c2": c2,
            "tri": tri, "id": idm, "w2": w2a,
        })
    return in_maps


def kernel(x, mask, index, rope, Wqkv_w, Wqkv_b, out_w, out_b,
           k_cache, v_cache):
    from concourse.bass_utils import run_bass_kernel_spmd

    x = np.asarray(x)
    rope = np.asarray(rope)
    Wqkv_w = np.asarray(Wqkv_w)
    Wqkv_b = np.asarray(Wqkv_b)
    out_w = np.asarray(out_w)
    out_b = np.asarray(out_b)

    if "nc" not in _CACHE:
        _CACHE["nc"] = _build_nc()
    nc = _CACHE["nc"]

    in_maps = _host_prep(x, rope, Wqkv_w, Wqkv_b, out_w)
    res = run_bass_kernel_spmd(nc, in_maps, core_ids=list(range(NCORES)))

    acc = np.zeros((NTOK, C), np.float32)
    for g in range(NCORES):
        acc += res.results[g]["out"].reshape(NTOK, C).astype(np.float32)
    # out bias + folded v-bias: sum_s p[s](v[s]+bv) = (sum p v) + bv
    bv = Wqkv_b[2 * C:3 * C].astype(np.float32)
    acc += out_b.astype(np.float32) + out_w.astype(np.float32) @ bv
    return acc.reshape(B, T, C)


# revision 18
# speedup vs baseline: 1.0368x; 1.0368x over previous
"""Trainium2 Bass kernel for nn_MHA_9603546874182.

Causal MHA: qkv proj + rope(32) + causal attention + out proj.
B=4, T=1024, C=2048, H=32, hd=64.

Sharding: 8-way tensor parallel over heads (4 heads / core).
Each core computes qkv for its 4 heads (column-parallel), rope,
causal attention, and a row-parallel partial of the output
projection (bf16 partials). Host sums the 8 partials (+ bias; the
v-bias is folded into the host-side output bias since
sum_s p[s] (v[s]+bv) = (sum p v) + bv after softmax).

Device pipeline (per core), software-pipelined across batches
(822us baseline -> 403us):
  tick (b, m8):
    phase1(b, m8):  v MMs (bf16) -> Vp;  q/k MMs -> +bias (DVE, fp32) ->
                    rope (DVE, bf16) -> PE-transpose pairs -> QT/KT (bf16)
    phase2:         one head-pair unit (2 heads on SBUF partition halves;
                    their K=64 score MMs run concurrently in different PE
                    row groups via auto tile_position). Scores^T skip the
                    fully-masked column block of diagonal-crossing s-tiles;
                    exp on ACT over live columns only; causal masking = one
                    triangular bf16 multiply on the 128-wide diagonal band
                    of the probabilities (DVE); AV+rowsum via [V|1] lhsT;
                    reciprocal_approx_fast (base-0 SBUF bounce: the custom
                    DVE op mis-addresses base_partition!=0 inputs) +
                    gpsimd partition_broadcast (sole gpsimd op kind --
                    mixing op kinds there costs a ~5.6us LOAD_LIB swap) +
                    normalize -> ctxT (bf16). qb0 units run inside their
                    own batch's phase1 ticks (they only need token tiles
                    0-3); qb1 units + all phase3 run in the next batch's
                    first 4 ticks, keeping the end-of-kernel drain short.
    phase3(bb, mm): out partial = ctxT.T @ w2 (bf16), psum -> DVE/ACT
                    copy (alternating) -> DRAM bf16
"""

import numpy as np

B, T, C, H = 4, 1024, 2048, 32
HD = C // H          # 64
NCORES = 8
HPC = H // NCORES    # 4 heads per core
SC = HPC * HD        # 256 shard channels
NTOK = B * T         # 4096
KT16 = C // 128      # 16 k tiles
MT = NTOK // 128     # 32 token tiles
MPB = T // 128       # 8 token tiles per batch
ROT = 32
NEG = -1.0e9

_CACHE = {}


def _build_nc():
    import concourse.bass as bass
    import concourse.mybir as mybir
    import concourse.tile as tile
    from concourse import bacc

    f32 = mybir.dt.float32
    f32r = mybir.dt.float32r
    bf16 = mybir.dt.bfloat16

    nc = bacc.Bacc("TRN2")

    xt_d = nc.dram_tensor("xt", [128, KT16, MT, 128], bf16, kind="ExternalInput")
    wq_d = nc.dram_tensor("wq", [128, KT16, 3 * SC], bf16, kind="ExternalInput")
    br_d = nc.dram_tensor("br", [128, 2 * SC], f32, kind="ExternalInput")
    c1_d = nc.dram_tensor("c1", [128, MPB, SC], bf16, kind="ExternalInput")
    c2_d = nc.dram_tensor("c2", [128, MPB, SC], bf16, kind="ExternalInput")
    tri_d = nc.dram_tensor("tri", [128, 128], bf16, kind="ExternalInput")
    id_d = nc.dram_tensor("id", [128, 128], bf16, kind="ExternalInput")
    w2_d = nc.dram_tensor("w2", [128, 2, C], bf16, kind="ExternalInput")
    out_d = nc.dram_tensor("out", [MT, 128, C], bf16, kind="ExternalOutput")

    with tile.TileContext(nc) as tc:
        with (
            tc.tile_pool(name="const", bufs=1) as const,
            tc.tile_pool(name="bigp", bufs=2) as bigp,
            tc.tile_pool(name="xp", bufs=4) as xp,
            tc.tile_pool(name="qkvp", bufs=4) as qkvp,
            tc.tile_pool(name="rtp", bufs=3) as rtp,
            tc.tile_pool(name="ptp", bufs=6) as ptp,
            tc.tile_pool(name="otp", bufs=3) as otp,
            tc.tile_pool(name="rsp", bufs=3) as rsp,
            tc.tile_pool(name="pqk", bufs=2, space="PSUM") as pqk,
            tc.tile_pool(name="pvb", bufs=1, space="PSUM") as pvb,
            tc.tile_pool(name="stp", bufs=3, space="PSUM") as stpp,
            tc.tile_pool(name="pcp", bufs=2, space="PSUM") as pcp,
        ):
            wq = const.tile([128, KT16, 3 * SC], bf16)
            nc.sync.dma_start(wq[:], wq_d[:])
            w2 = const.tile([128, 2, C], bf16)
            nc.sync.dma_start(w2[:], w2_d[:])
            br = const.tile([128, 2 * SC], f32)
            nc.sync.dma_start(br[:], br_d[:])
            c1 = const.tile([128, MPB, SC], bf16)
            nc.sync.dma_start(c1[:], c1_d[:])
            c2 = const.tile([128, MPB, SC], bf16)
            nc.sync.dma_start(c2[:], c2_d[:])
            tri = const.tile([128, 128], bf16)
            nc.sync.dma_start(tri[:], tri_d[:])
            ident = const.tile([128, 128], bf16)
            nc.sync.dma_start(ident[:], id_d[:])

            # Per-batch double-buffered state
            QTs, KTs, Vps, CTs = {}, {}, {}, {}

            def get_batch_tiles(b):
                if b not in QTs:
                    QTs[b] = bigp.tile([128, 2, T], bf16, tag="qt",
                                       name=f"qt{b}")
                    KTs[b] = bigp.tile([128, 2, T], bf16, tag="kt",
                                       name=f"kt{b}")
                    Vps[b] = bigp.tile([128, MPB, HPC, HD + 1], bf16,
                                       tag="vp", name=f"vp{b}")
                    CTs[b] = bigp.tile([128, 2, T], bf16, tag="ct",
                                       name=f"ct{b}")
                    nc.vector.memset(Vps[b][:, :, :, HD:HD + 1], 1.0)
                return QTs[b], KTs[b], Vps[b], CTs[b]

            def phase1(b, m8):
                QT, KTt, Vp, _ = get_batch_tiles(b)
                m = b * MPB + m8
                xt = xp.tile([128, KT16, 128], bf16)
                nc.sync.dma_start(xt[:], xt_d[:, :, m, :])
                # v matmuls (no bias: folded into host output bias)
                psB = pvb.tile([128, SC], f32, tag="vtr", name="psB")
                for k in range(KT16):
                    nc.tensor.matmul(
                        psB[:], xt[:, k, :], wq[:, k, 2 * SC:3 * SC],
                        start=(k == 0), stop=(k == KT16 - 1))
                nc.scalar.copy(Vp[:, m8, :, 0:HD], psB[:].rearrange(
                    "p (h d) -> p h d", h=HPC))
                # q/k matmuls
                psA = pqk.tile([128, 512], f32)
                for k in range(KT16):
                    nc.tensor.matmul(
                        psA[:], xt[:, k, :], wq[:, k, 0:512],
                        start=(k == 0), stop=(k == KT16 - 1))
                qkv = qkvp.tile([128, 512], bf16)
                nc.vector.tensor_add(qkv[:], psA[:], br[:])
                # rope on DVE (bf16 2x mode; gpsimd is reserved for
                # partition_broadcast — mixing op kinds there causes a
                # ~5.6us LOAD_LIB stall per switch)
                c1v = c1[:, m8, :].rearrange("p (h d) -> p h d", h=HPC)
                c2v = c2[:, m8, :].rearrange("p (h d) -> p h d", h=HPC)
                for base in (0, 256):
                    sec = qkv[:, base:base + 256].rearrange(
                        "p (h d) -> p h d", h=HPC)
                    rt = rtp.tile([128, 256], bf16)
                    rtv = rt.rearrange("p (h d) -> p h d", h=HPC)
                    nc.vector.tensor_mul(
                        rtv[:, :, 0:16], sec[:, :, 16:32], c2v[:, :, 0:16])
                    nc.vector.tensor_mul(
                        rtv[:, :, 16:32], sec[:, :, 0:16], c2v[:, :, 16:32])
                    nc.vector.tensor_mul(sec[:], sec[:], c1v)
                    nc.vector.tensor_add(
                        sec[:, :, 0:ROT], sec[:, :, 0:ROT], rtv[:, :, 0:ROT])
                # transpose q/k -> QT/KT (bf16): all four 128-chunks in
                # one psum tile (fewer allocations on the shared bank),
                # one copy per destination
                tp = pvb.tile([128, 4, 128], bf16, tag="vtr", name="tp")
                for ci in range(4):
                    nc.tensor.transpose(
                        tp[:, ci, :], qkv[:, ci * 128:(ci + 1) * 128], ident)
                nc.vector.tensor_copy(
                    QT[:, 0:2, m8 * 128:(m8 + 1) * 128], tp[:, 0:2, :])
                nc.vector.tensor_copy(
                    KTt[:, 0:2, m8 * 128:(m8 + 1) * 128], tp[:, 2:4, :])

            def phase2(bb, u):
                QT, KTt, Vp, ctxT = get_batch_tiles(bb)
                qb, j = u // 2, u % 2
                hA, hB = 2 * j, 2 * j + 1
                nst = 4 * (qb + 1)
                qsl = slice(qb * 512, (qb + 1) * 512)
                pctA = pcp.tile([HD + 1, 512], f32, tag="pc")
                pctB = pcp.tile([HD + 1, 512], f32, tag="pc")
                for st in range(nst):
                    r = st - 4 * qb
                    # columns [0, 128r) of this s-tile are fully in the
                    # future for every s row — skip them entirely; only the
                    # diagonal 128-band needs the triangular keep-mask
                    lo = 128 * r if r > 0 else 0
                    stA = stpp.tile([128, 512], f32, tag="st")
                    stB = stpp.tile([128, 512], f32, tag="st")
                    ssl = slice(st * 128, (st + 1) * 128)
                    qls = slice(qb * 512 + lo, (qb + 1) * 512)
                    nc.tensor.matmul(
                        stA[:, lo:512], KTt[0:64, j, ssl], QT[0:64, j, qls],
                        start=True, stop=True)
                    nc.tensor.matmul(
                        stB[:, lo:512], KTt[64:128, j, ssl], QT[64:128, j, qls],
                        start=True, stop=True)
                    ptA = ptp.tile([128, 512], bf16, tag="pt")
                    ptB = ptp.tile([128, 512], bf16, tag="pt")
                    nc.scalar.activation(
                        ptA[:, lo:512], stA[:, lo:512],
                        mybir.ActivationFunctionType.Exp)
                    nc.scalar.activation(
                        ptB[:, lo:512], stB[:, lo:512],
                        mybir.ActivationFunctionType.Exp)
                    if r >= 0:
                        nc.vector.tensor_mul(
                            ptA[:, lo:lo + 128], ptA[:, lo:lo + 128], tri[:])
                        nc.vector.tensor_mul(
                            ptB[:, lo:lo + 128], ptB[:, lo:lo + 128], tri[:])
                    nc.tensor.matmul(
                        pctA[:, lo:512], Vp[:, st, hA, :], ptA[:, lo:512],
                        start=(st == 0), stop=(st == nst - 1),
                        skip_group_check=True)
                    nc.tensor.matmul(
                        pctB[:, lo:512], Vp[:, st, hB, :], ptB[:, lo:512],
                        start=(st == 0), stop=(st == nst - 1),
                        skip_group_check=True)
                # software-pipeline the two heads' normalize chains so
                # head B's psum bank releases as early as head A's.
                # (reciprocal_approx_fast mis-addresses base_partition!=0
                # inputs — bounce the rowsum row to a base-0 SBUF tile.)
                pcts = ((pctA, 0), (pctB, 64))
                rs0s, rss, rsbs = [], [], []
                for pct, p0 in pcts:
                    rs0 = rsp.tile([1, 512], f32, tag="rs0")
                    nc.vector.tensor_copy(rs0[:], pct[HD:HD + 1, :])
                    rs0s.append(rs0)
                for i in range(2):
                    rs = rsp.tile([1, 512], f32, tag="rs")
                    nc.vector.reciprocal_approx_fast(rs[:], rs0s[i][:])
                    rss.append(rs)
                for i in range(2):
                    rsb = rsp.tile([HD, 512], f32, tag="rsb")
                    nc.gpsimd.partition_broadcast(rsb[:], rss[i][:])
                    rsbs.append(rsb)
                for i, (pct, p0) in enumerate(pcts):
                    nc.vector.tensor_mul(
                        ctxT[p0:p0 + 64, j, qsl], pct[0:HD, :], rsbs[i][:])

            def phase3(bb, mm):
                _, _, _, ctxT = get_batch_tiles(bb)
                m = bb * MPB + mm
                msl = slice(mm * 128, (mm + 1) * 128)
                ot = otp.tile([128, C], bf16)
                for n in range(4):
                    po = pcp.tile([128, 512], f32, tag="pc")
                    for jj in range(2):
                        nc.tensor.matmul(
                            po[:], ctxT[:, jj, msl],
                            w2[:, jj, n * 512:(n + 1) * 512],
                            start=(jj == 0), stop=(jj == 1))
                    if n % 2 == 0:
                        nc.vector.tensor_copy(ot[:, n * 512:(n + 1) * 512], po[:])
                    else:
                        nc.scalar.copy(ot[:, n * 512:(n + 1) * 512], po[:])
                nc.sync.dma_start(out_d[m, :, :], ot[:])

            # qb0 units only need the first 4 token tiles of their own
            # batch -> run them inside batch b's phase1 ticks; qb1 units and
            # all phase3 chunks run in the first 4 ticks of the next batch,
            # shrinking the end-of-kernel drain to 4 ticks.
            for tick in range(B * MPB + 4):
                b, m8 = divmod(tick, MPB)
                if b < B:
                    phase1(b, m8)
                    if m8 == 5:
                        phase2(b, 0)
                    elif m8 == 6:
                        phase2(b, 1)
                if 1 <= b <= B and m8 in (0, 1):
                    phase2(b - 1, 2 + m8)
                if 1 <= b <= B and m8 < 4:
                    phase3(b - 1, 2 * m8)
                    phase3(b - 1, 2 * m8 + 1)

    nc.finalize()
    return nc


def _host_prep(x, rope, Wqkv_w, Wqkv_b, out_w):
    """Build per-core input maps (bf16 partition-first layouts)."""
    import ml_dtypes
    bf16 = ml_dtypes.bfloat16

    xf = np.ascontiguousarray(x.reshape(NTOK, C)).astype(np.float32)
    # xt[p, k, m, t] = x[m*128+t, k*128+p]
    xt = np.ascontiguousarray(
        xf.reshape(MT, 128, KT16, 128).transpose(3, 2, 0, 1)).astype(bf16)

    # rope tables (position within a batch: t = 0..1023)
    cos = rope[:, :, 0].astype(np.float32)   # [T, 16]
    sin = rope[:, :, 1].astype(np.float32)
    C1h = np.ones((T, HD), np.float32)
    C1h[:, 0:16] = cos
    C1h[:, 16:32] = cos
    C2h = np.zeros((T, HD), np.float32)
    C2h[:, 0:16] = -sin
    C2h[:, 16:32] = sin
    C1 = np.tile(C1h, (1, HPC))              # [T, 256]
    C2 = np.tile(C2h, (1, HPC))
    # c1[p, q, j] = C1[q*128+p, j]
    c1 = np.ascontiguousarray(
        C1.reshape(MPB, 128, SC).transpose(1, 0, 2)).astype(bf16)
    c2 = np.ascontiguousarray(
        C2.reshape(MPB, 128, SC).transpose(1, 0, 2)).astype(bf16)

    # triangular keep-mask for the diagonal 128-band: tri[p, y] = (y >= p)
    yy = np.arange(128)[None, :]
    pp = np.arange(128)[:, None]
    tri = (yy >= pp).astype(bf16)
    idm = np.eye(128, dtype=np.float32).astype(bf16)

    scale = np.float32(1.0 / np.sqrt(HD))
    in_maps = []
    for g in range(NCORES):
        hs = g * SC
        Wq = Wqkv_w[hs:hs + SC, :].astype(np.float32) * scale
        Wk = Wqkv_w[C + hs:C + hs + SC, :].astype(np.float32)
        Wv = Wqkv_w[2 * C + hs:2 * C + hs + SC, :].astype(np.float32)
        Wsh = np.concatenate([Wq, Wk, Wv], axis=0)          # [768, 2048]
        # wq[p, k, j] = Wsh[j, k*128+p]
        wqa = np.ascontiguousarray(
            Wsh.T.reshape(KT16, 128, 3 * SC).transpose(1, 0, 2)).astype(bf16)
        bq = Wqkv_b[hs:hs + SC].astype(np.float32) * scale
        bk = Wqkv_b[C + hs:C + hs + SC].astype(np.float32)
        bsh = np.concatenate([bq, bk])
        bra = np.ascontiguousarray(
            np.broadcast_to(bsh, (128, 2 * SC))).astype(np.float32)
        # w2[p, j, o] = out_w[o, g*256 + j*128 + p]
        w2a = np.ascontiguousarray(
            out_w[:, hs:hs + SC].astype(np.float32).T.reshape(
                2, 128, C).transpose(1, 0, 2)).astype(bf16)
        in_maps.append({
            "xt": xt, "wq": wqa, "br": bra, "c1": c1, "c2": c2,
            "tri": tri, "id": idm, "w2": w2a,
        })
    return in_maps


def kernel(x, mask, index, rope, Wqkv_w, Wqkv_b, out_w, out_b,
           k_cache, v_cache):
    from concourse.bass_utils import run_bass_kernel_spmd

    x = np.asarray(x)
    rope = np.asarray(rope)
    Wqkv_w = np.asarray(Wqkv_w)
    Wqkv_b = np.asarray(Wqkv_b)
    out_w = np.asarray(out_w)
    out_b = np.asarray(out_b)

    if "nc" not in _CACHE:
        _CACHE["nc"] = _build_nc()
    nc = _CACHE["nc"]

    in_maps = _host_prep(x, rope, Wqkv_w, Wqkv_b, out_w)
    res = run_bass_kernel_spmd(nc, in_maps, core_ids=list(range(NCORES)))

    acc = np.zeros((NTOK, C), np.float32)
    for g in range(NCORES):
        acc += res.results[g]["out"].reshape(NTOK, C).astype(np.float32)
    # out bias + folded v-bias: sum_s p[s](v[s]+bv) = (sum p v) + bv
    bv = Wqkv_b[2 * C:3 * C].astype(np.float32)
    acc += out_b.astype(np.float32) + out_w.astype(np.float32) @ bv
    return acc.reshape(B, T, C)
